# revision 1
# baseline (speedup 1.0000x reference)
"""Associative-embedding (push/pull) loss on 8 TRN2 NeuronCores.

Strategy (data parallel, 8 images per core):
  - The 285MB tags tensor is only touched at P*K=510 points per image, and
    only ~1/4 of those are valid (joint_img_valid & person_valid). Each
    core gathers ONLY its valid points (~1020), host-sorted by address and
    packed into C indirect-DMA window calls of 128 single-element
    descriptors (the HW contract: one index per destination partition).
    C=9 covers the valid count with >5 sigma margin; a C=32 variant
    (capacity 4096 >= all 4080 points) is compiled lazily as fallback.
  - Per-point moment contributions v, v^2 are scattered into per-image
    columns with host-built 0/1 matrices (two DVE ops per call) and
    accumulated into a PSUM [30 person, 16] tile by one PE matmul per call
    against a host-built point->person selection matrix. This replaces all
    on-device masking/reduction; everything derivable from the masks alone
    (1/cnt, per-image normalizers, invalid-person fake-mean offsets) is
    precomputed on the host in one small aux upload.
  - The pairwise push term runs in an [img part, person] layout after a
    32x32 DVE block transpose, using an invalid-person fake-mean trick so
    no pair mask is needed: push_sum = sum_ij exp(-(m'_i-m'_j)^2) - P.
    exp(-d^2) + its row sum run on the scalar engine (Square, then Exp
    with the fused accumulator).
  - Each core emits (push, pull) partials already scaled by 1/64; the host
    sums the 8 partials.
"""

import sys

import numpy as np

if "/opt/trn_rl_repo" not in sys.path:
    sys.path.insert(0, "/opt/trn_rl_repo")

from concourse import bacc, bass, mybir, tile  # noqa: E402
from concourse import bass_utils  # noqa: E402

B, P, K, H, W = 64, 30, 17, 256, 256
NCORES = 8
BPC = B // NCORES           # 8 images per core
J = BPC * K                 # 136 (img, k) columns
KHW = K * H * W
NTOT = BPC * KHW            # flat tag elements per core

C_FAST = 9                  # 1152-point capacity (valid ~1020 +- 28)
C_FULL = 32                 # 4096-point capacity (any input)

# aux30 [30, 26] f32 columns: 0:8 inv, 8:16 fakeA, 16:24 IC3,
#   24 c1 (rows 0:8), 25 c2 (rows 0:8)
AUX30W = 26

f32 = mybir.dt.float32
i32 = mybir.dt.int32
Alu = mybir.AluOpType
Act = mybir.ActivationFunctionType
AX = mybir.AxisListType


def build_nc(ncalls=C_FAST):
    nc = bacc.Bacc("TRN2", target_bir_lowering=False, debug=False,
                   num_devices=NCORES)

    tags = nc.dram_tensor("tags", [NTOT, 1], f32, kind="ExternalInput")
    idx_in = nc.dram_tensor("idx", [128, ncalls], i32, kind="ExternalInput")
    # fmat columns: VA [16*ncalls] (v -> col img), VB [16*ncalls]
    # (v^2 -> col 8+img), PSEL [30*ncalls] (point -> person)
    fmat_in = nc.dram_tensor("fmat", [128, 62 * ncalls], f32,
                             kind="ExternalInput")
    aux_in = nc.dram_tensor("aux", [P, AUX30W], f32, kind="ExternalInput")
    out = nc.dram_tensor("out", [1, 2], f32, kind="ExternalOutput")

    va_off, vb_off, ps_off = 0, 16 * ncalls, 32 * ncalls

    with tile.TileContext(nc) as tc:
        with tc.tile_pool(name="sbuf", bufs=1) as pool, \
             tc.tile_pool(name="psum", bufs=1, space="PSUM") as psp:

            idxm = pool.tile([128, ncalls], i32)
            fmat = pool.tile([128, 62 * ncalls], f32)
            aux = pool.tile([P, AUX30W], f32)
            nc.sync.dma_start(out=idxm[:], in_=idx_in[:])
            nc.sync.dma_start(out=fmat[:], in_=fmat_in[:])
            nc.sync.dma_start(out=aux[:], in_=aux_in[:])

            inv = aux[:, 0:BPC]
            fakeA = aux[:, BPC:2 * BPC]
            ic3 = aux[:, 2 * BPC:3 * BPC]
            c1 = aux[0:BPC, 24:25]
            c2 = aux[0:BPC, 25:26]

            ones = pool.tile([P, 1], f32)
            nc.vector.memset(ones[:], 1.0)
            t_in = pool.tile([32, 32], f32)
            nc.vector.memset(t_in[:], 0.0)

            # ---- gather valid points + scatter-accumulate moments ----
            v4 = pool.tile([128, ncalls, 1], f32)
            rhs = pool.tile([128, ncalls, 16], f32)
            ps = psp.tile([P, 16], f32)
            for c in range(ncalls):
                nc.gpsimd.indirect_dma_start(
                    out=v4[:, c, :], out_offset=None, in_=tags[:],
                    in_offset=bass.IndirectOffsetOnAxis(ap=idxm[:, c:c + 1],
                                                        axis=0))
            for c in range(ncalls):
                v_bc = v4[:, c, :].to_broadcast([128, 16])
                rc = rhs[:, c, :]
                # rhs = (VB*v + VA) * v  ->  v at col img, v^2 at col 8+img
                nc.vector.tensor_tensor(
                    out=rc, in0=fmat[:, vb_off + 16 * c:vb_off + 16 * c + 16],
                    in1=v_bc, op=Alu.mult)
                nc.vector.tensor_tensor(
                    out=rc, in0=rc,
                    in1=fmat[:, va_off + 16 * c:va_off + 16 * c + 16],
                    op=Alu.add)
                nc.vector.tensor_tensor(out=rc, in0=rc, in1=v_bc, op=Alu.mult)
                nc.tensor.matmul(
                    out=ps[:],
                    lhsT=fmat[:, ps_off + 30 * c:ps_off + 30 * c + 30],
                    rhs=rc, start=(c == 0), stop=(c == ncalls - 1))

            s12 = pool.tile([P, 16], f32)
            nc.vector.tensor_copy(out=s12[:], in_=ps[:])
            s1 = s12[:, 0:BPC]
            s2 = s12[:, BPC:2 * BPC]

            # ---- means + fake-mean for invalid persons ----
            mean = pool.tile([P, BPC], f32)
            nc.vector.tensor_tensor(out=mean[:], in0=s1, in1=inv,
                                    op=Alu.mult)
            nc.vector.tensor_tensor(out=t_in[0:P, 0:BPC], in0=mean[:],
                                    in1=fakeA, op=Alu.add)
            t_out = pool.tile([32, 32], f32)
            nc.vector.transpose(out=t_out[:], in_=t_in[:])
            meanT = t_out[0:BPC, 0:P]          # [8, 30]

            # ---- push: s_acc[img] = sum_ij exp(-(m'_i - m'_j)^2) ----
            d = pool.tile([BPC, P, P], f32)
            nc.vector.tensor_tensor(
                out=d[:],
                in0=meanT.unsqueeze(2).to_broadcast([BPC, P, P]),
                in1=meanT.unsqueeze(1).to_broadcast([BPC, P, P]),
                op=Alu.subtract)
            sq = pool.tile([BPC, P, P], f32)
            e = pool.tile([BPC, P, P], f32)
            s_acc = pool.tile([BPC, 1], f32)
            nc.scalar.activation(out=sq[:], in_=d[:], func=Act.Square,
                                 scale=1.0)
            nc.scalar.activation(out=e[:], in_=sq[:], func=Act.Exp,
                                 scale=-1.0, accum_out=s_acc[:])

            # ---- pull: pw = (s2 - s1*mean) * inv*ninv/B ----
            sm = pool.tile([P, BPC], f32)
            dd = pool.tile([P, BPC], f32)
            pw = pool.tile([P, BPC], f32)
            pwr = pool.tile([P, 1], f32)
            nc.vector.tensor_tensor(out=sm[:], in0=s1, in1=mean[:],
                                    op=Alu.mult)
            nc.vector.tensor_tensor(out=dd[:], in0=s2, in1=sm[:],
                                    op=Alu.subtract)
            nc.vector.tensor_tensor(out=pw[:], in0=dd[:], in1=ic3,
                                    op=Alu.mult)
            nc.vector.tensor_reduce(out=pwr[:], in_=pw[:], axis=AX.X,
                                    op=Alu.add)

            # push_img = s_acc*c1 - c2  (c1 = 0.5*g/den/B, c2 = P*c1)
            pp0 = pool.tile([BPC, 1], f32)
            nc.vector.tensor_tensor(out=pp0[:], in0=s_acc[:], in1=c1,
                                    op=Alu.mult)
            nc.vector.tensor_tensor(out=pp0[:], in0=pp0[:], in1=c2,
                                    op=Alu.subtract)

            # ---- final sums: psum[0,0]=push, psum[0,1]=pull ----
            acc = psp.tile([1, 2], f32)
            nc.tensor.matmul(out=acc[:, 0:1], lhsT=pp0[:], rhs=ones[0:BPC, :],
                             start=True, stop=True)
            nc.tensor.matmul(out=acc[:, 1:2], lhsT=pwr[:], rhs=ones[:],
                             start=True, stop=True)
            res = pool.tile([1, 2], f32)
            nc.vector.tensor_copy(out=res[:], in_=acc[:])
            nc.sync.dma_start(out=out[:], in_=res[:])

    nc.compile()
    return nc


_nc_cache = {}


def _get_nc(ncalls=C_FAST):
    if ncalls not in _nc_cache:
        _nc_cache[ncalls] = build_nc(ncalls)
    return _nc_cache[ncalls]


def make_in_maps(tags, joints, jv, pv, ncalls=None):
    """Host preprocessing: per-core input dict. Returns (in_maps, ncalls)."""
    tags = np.ascontiguousarray(np.asarray(tags, dtype=np.float32))
    joints = np.asarray(joints, dtype=np.int64)
    jv = np.asarray(jv)
    pv = np.asarray(pv)

    m_all = (jv > 0) & (pv[:, :, None] > 0)            # [64, 30, 17]
    if ncalls is None:
        nv_max = max(int(m_all[c * BPC:(c + 1) * BPC].sum())
                     for c in range(NCORES))
        ncalls = C_FAST if nv_max <= 128 * C_FAST else C_FULL

    in_maps = []
    for c in range(NCORES):
        sl = slice(c * BPC, (c + 1) * BPC)
        m = m_all[sl]                                   # [8, 30, 17]
        x = joints[sl, :, :, 0]
        y = joints[sl, :, :, 1]
        img_i, p_i, k_i = np.nonzero(m)
        idx_v = (65536 * (img_i * K + k_i) + 256 * x[img_i, p_i, k_i]
                 + y[img_i, p_i, k_i]).astype(np.int64)
        order = np.argsort(idx_v, kind="stable")
        idx_v, img_i, p_i = idx_v[order], img_i[order], p_i[order]
        nv = idx_v.shape[0]
        assert nv <= 128 * ncalls, (nv, ncalls)

        t = np.arange(nv)
        q_t, c_t = t % 128, t // 128
        idxm = np.zeros((128, ncalls), dtype=np.int32)
        idxm[q_t, c_t] = idx_v
        fmat = np.zeros((128, 62 * ncalls), dtype=np.float32)
        fmat[q_t, 16 * c_t + img_i] = 1.0                       # VA
        fmat[q_t, 16 * ncalls + 16 * c_t + 8 + img_i] = 1.0     # VB
        fmat[q_t, 32 * ncalls + 30 * c_t + p_i] = 1.0           # PSEL

        cnt = m.sum(axis=2).T.astype(np.float32)        # [30, 8]
        inv = 1.0 / np.maximum(cnt, 1.0)
        fake = (cnt <= 0) * (1000.0 * (np.arange(P) + 1.0))[:, None]
        n = (cnt > 0).sum(axis=0)                       # [8]
        den = np.maximum(n * (n - 1.0), 1.0)
        c1 = 0.5 * (n > 1) / den / B
        ninv = 1.0 / np.maximum(n, 1.0)
        ic3 = inv * (ninv / B)[None, :]

        aux = np.zeros((P, AUX30W), dtype=np.float32)
        aux[:, 0:BPC] = inv
        aux[:, BPC:2 * BPC] = fake
        aux[:, 2 * BPC:3 * BPC] = ic3
        aux[0:BPC, 24] = c1
        aux[0:BPC, 25] = P * c1

        in_maps.append({
            "tags": tags[sl].reshape(NTOT, 1),
            "idx": idxm,
            "fmat": fmat,
            "aux": aux,
        })
    return in_maps, ncalls


def kernel(tags, joints, joint_img_valid, person_valid):
    in_maps, ncalls = make_in_maps(tags, joints, joint_img_valid,
                                   person_valid)
    nc = _get_nc(ncalls)
    res = bass_utils.run_bass_kernel_spmd(nc, in_maps,
                                          core_ids=list(range(NCORES)))
    outs = [np.asarray(r["out"], dtype=np.float64).reshape(2)
            for r in res.results]
    total = np.sum(outs, axis=0)
    return np.float32(total[0]), np.float32(total[1])


if __name__ == "__main__":
    rng = np.random.default_rng(0)
    t = rng.standard_normal((B, K, H, W), dtype=np.float32)
    j = rng.integers(0, H, size=(B, P, K, 2), dtype=np.int32)
    jv_ = rng.integers(0, 2, size=(B, P, K), dtype=np.int32)
    pv_ = rng.integers(0, 2, size=(B, P), dtype=np.int32)
    print(kernel(t, j, jv_, pv_))

